# revision 2
# baseline (speedup 1.0000x reference)
"""GraphSAGE (2-level mean-aggregate) Trainium2 Bass kernel.

Math (reference simplification):
  agg0[b] = mean_n0 h1[b,n0]  where h1 = [e_n | agg1] @ W1
          = (mean_n0 e_n[b]) @ W1[:64] + (mean_{n0,n1} e_nn[b]) @ W1[64:]
  out[b]  = sigmoid(e_v[b] @ W0[:64] + agg0[b] @ W0[64:] + b0)

So per batch row we only need three 64-d vectors: the row's own embedding,
the mean of its 10 neigh0 embeddings, and the mean of all 250 neigh1
embeddings.  Everything else is tiny dense algebra.

Distribution: data-parallel over batch across 8 cores (512 rows/core), the
1M-row embedding table replicated per core.  The dominant cost is the
irregular gather: 512*261 = 133,632 rows per core via gpsimd indirect DMA
(one 64-elem row per partition per instruction).  The gather is
SWDGE-descriptor-generation + drain bound, so the table is stored as
float8_e4m3 (64 B/row instead of 256 B) to cut DMA drain time and SBUF
pressure; the segmented sums run on DVE with f32 accumulation, so the only
precision loss is per-element fp8 rounding of table entries (max rel err
~2.5e-3 at the sigmoid output, well inside the 2e-2 gate).
"""

import os

import numpy as np
import ml_dtypes

import concourse.bass as bass
import concourse.mybir as mybir
from concourse import bacc
from concourse.bass_utils import run_bass_kernel_spmd
from concourse.masks import make_identity
from concourse.tile import TileContext

N_CORES = 8
B = 4096
BPC = B // N_CORES          # 512 batch rows per core
CHUNK = 128                 # batch rows per chunk (= SBUF partitions)
NCHUNK = BPC // CHUNK       # 4
N0 = 10
NN1 = 250                   # 10 * 25 flattened neigh1 per row
K1 = 125                    # neigh1 gather split: 2 gathers of 125 rows/partition
D = 64
H1 = 128
H0 = 128
VOCAB = 1_000_001
IDXW = NN1 + N0 + 1         # 261 indices per batch row

_prog_cache = {}


def _build_program():
    nc = bacc.Bacc()
    f32 = mybir.dt.float32
    fp8 = mybir.dt.float8e4
    i32 = mybir.dt.int32

    table = nc.declare_dram_parameter("table", [VOCAB, D], fp8, isOutput=False)
    idx = nc.declare_dram_parameter("idx", [CHUNK, NCHUNK * IDXW], i32, isOutput=False)
    w1 = nc.declare_dram_parameter("w1", [2 * D, H1], f32, isOutput=False)
    w0 = nc.declare_dram_parameter("w0", [D + H1, H0], f32, isOutput=False)
    b0 = nc.declare_dram_parameter("b0", [1, H0], f32, isOutput=False)
    out = nc.declare_dram_parameter("out", [BPC, H0], f32, isOutput=True)

    AX = mybir.AxisListType
    ALU = mybir.AluOpType
    AF = mybir.ActivationFunctionType

    with TileContext(nc) as tc:
        with (
            tc.tile_pool(name="const", bufs=1) as cp,
            tc.tile_pool(name="g1", bufs=3) as g1p,
            tc.tile_pool(name="g0", bufs=2) as g0p,
            tc.tile_pool(name="sm", bufs=2) as sp,
            tc.tile_pool(name="ps", bufs=2, space="PSUM") as pp,
        ):
            ident = cp.tile([128, 128], f32)
            make_identity(nc, ident[:])
            ones1 = cp.tile([1, CHUNK], f32)
            nc.gpsimd.memset(ones1[:], 1.0)

            w1a_sb = cp.tile([D, H1], f32)
            nc.sync.dma_start(out=w1a_sb[:], in_=w1[0:D, :])
            w1b_sb = cp.tile([D, H1], f32)
            nc.sync.dma_start(out=w1b_sb[:], in_=w1[D : 2 * D, :])
            w0e_sb = cp.tile([D, H0], f32)
            nc.sync.dma_start(out=w0e_sb[:], in_=w0[0:D, :])
            w0a_sb = cp.tile([H1, H0], f32)
            nc.sync.dma_start(out=w0a_sb[:], in_=w0[D : D + H1, :])
            b0_sb = cp.tile([1, H0], f32)
            nc.sync.dma_start(out=b0_sb[:], in_=b0[:])
            idx_sb = cp.tile([CHUNK, NCHUNK * IDXW], i32)
            nc.sync.dma_start(out=idx_sb[:], in_=idx[:])

            for c in range(NCHUNK):
                base = c * IDXW

                # ---- irregular gathers (partition p = batch row c*128+p) ----
                # HW indirect DMA honors ONE offset per partition per
                # instruction, so each instruction gathers 128 rows (one
                # 64-elem fp8 row per partition) into a 64-col slice.
                g1a = g1p.tile([CHUNK, K1 * D], fp8, tag="g1")
                g1b = g1p.tile([CHUNK, K1 * D], fp8, tag="g1")
                for j in range(K1):
                    nc.gpsimd.indirect_dma_start(
                        out=g1a[:, j * D : (j + 1) * D],
                        out_offset=None,
                        in_=table[:],
                        in_offset=bass.IndirectOffsetOnAxis(
                            ap=idx_sb[:, base + j : base + j + 1], axis=0
                        ),
                    )
                    nc.gpsimd.indirect_dma_start(
                        out=g1b[:, j * D : (j + 1) * D],
                        out_offset=None,
                        in_=table[:],
                        in_offset=bass.IndirectOffsetOnAxis(
                            ap=idx_sb[:, base + K1 + j : base + K1 + j + 1], axis=0
                        ),
                    )
                g0 = g0p.tile([CHUNK, N0 * D], fp8, tag="g0")
                for j in range(N0):
                    nc.gpsimd.indirect_dma_start(
                        out=g0[:, j * D : (j + 1) * D],
                        out_offset=None,
                        in_=table[:],
                        in_offset=bass.IndirectOffsetOnAxis(
                            ap=idx_sb[:, base + NN1 + j : base + NN1 + j + 1], axis=0
                        ),
                    )
                ev8 = sp.tile([CHUNK, D], fp8, tag="ev8")
                nc.gpsimd.indirect_dma_start(
                    out=ev8[:],
                    out_offset=None,
                    in_=table[:],
                    in_offset=bass.IndirectOffsetOnAxis(
                        ap=idx_sb[:, base + NN1 + N0 : base + IDXW], axis=0
                    ),
                )

                # ---- segmented sums (DVE, f32 accumulate): axis = neighbor ----
                s1p = sp.tile([CHUNK, 2 * D], f32, tag="s1p")
                nc.vector.tensor_reduce(
                    out=s1p[:, 0:D],
                    in_=g1a[:].rearrange("p (k d) -> p d k", d=D),
                    axis=AX.X,
                    op=ALU.add,
                )
                nc.vector.tensor_reduce(
                    out=s1p[:, D : 2 * D],
                    in_=g1b[:].rearrange("p (k d) -> p d k", d=D),
                    axis=AX.X,
                    op=ALU.add,
                )
                s1 = sp.tile([CHUNK, D], f32, tag="s1")
                nc.vector.tensor_add(
                    out=s1[:], in0=s1p[:, 0:D], in1=s1p[:, D : 2 * D]
                )
                s0 = sp.tile([CHUNK, D], f32, tag="s0")
                nc.vector.tensor_reduce(
                    out=s0[:],
                    in_=g0[:].rearrange("p (k d) -> p d k", d=D),
                    axis=AX.X,
                    op=ALU.add,
                )
                ev = sp.tile([CHUNK, D], f32, tag="ev")
                nc.vector.tensor_copy(out=ev[:], in_=ev8[:])

                # ---- transpose [128b, 64d] -> [64d, 128b] via PE ----
                s1t_ps = pp.tile([D, CHUNK], f32, tag="tp")
                nc.tensor.transpose(out=s1t_ps[:], in_=s1[:], identity=ident[:])
                s0t_ps = pp.tile([D, CHUNK], f32, tag="tp")
                nc.tensor.transpose(out=s0t_ps[:], in_=s0[:], identity=ident[:])
                evt_ps = pp.tile([D, CHUNK], f32, tag="tp")
                nc.tensor.transpose(out=evt_ps[:], in_=ev[:], identity=ident[:])

                s1t = sp.tile([D, CHUNK], f32, tag="s1t")
                nc.scalar.activation(
                    out=s1t[:], in_=s1t_ps[:], func=AF.Copy, scale=1.0 / NN1
                )
                s0t = sp.tile([D, CHUNK], f32, tag="s0t")
                nc.scalar.activation(
                    out=s0t[:], in_=s0t_ps[:], func=AF.Copy, scale=1.0 / N0
                )
                evt = sp.tile([D, CHUNK], f32, tag="evt")
                nc.scalar.activation(out=evt[:], in_=evt_ps[:], func=AF.Copy)

                # ---- A^T[h1, b] = W1a^T @ (S0^T/10) + W1b^T @ (S1^T/250) ----
                a_ps = pp.tile([H1, CHUNK], f32, tag="aps")
                nc.tensor.matmul(
                    out=a_ps[:], lhsT=w1a_sb[:], rhs=s0t[:], start=True, stop=False
                )
                nc.tensor.matmul(
                    out=a_ps[:], lhsT=w1b_sb[:], rhs=s1t[:], start=False, stop=True
                )
                at = sp.tile([H1, CHUNK], f32, tag="at")
                nc.vector.tensor_copy(out=at[:], in_=a_ps[:])

                # ---- O[b, h0] = EV @ W0e + A @ W0a + 1 x b0; sigmoid ----
                o_ps = pp.tile([CHUNK, H0], f32, tag="ops")
                nc.tensor.matmul(
                    out=o_ps[:], lhsT=evt[:], rhs=w0e_sb[:], start=True, stop=False
                )
                nc.tensor.matmul(
                    out=o_ps[:], lhsT=at[:], rhs=w0a_sb[:], start=False, stop=False
                )
                nc.tensor.matmul(
                    out=o_ps[:], lhsT=ones1[:], rhs=b0_sb[:], start=False, stop=True
                )
                ob = sp.tile([CHUNK, H0], f32, tag="ob")
                nc.scalar.activation(out=ob[:], in_=o_ps[:], func=AF.Sigmoid)
                nc.sync.dma_start(
                    out=out[c * CHUNK : (c + 1) * CHUNK, :], in_=ob[:]
                )

    nc.finalize()
    return nc


def _make_idx(inputs, neigh0, neigh1, core):
    """Per-core index tensor [128, NCHUNK*261], partition p = batch row
    within chunk; per chunk the 261 columns are [250 x neigh1 | 10 x
    neigh0 | input]."""
    rows = slice(core * BPC, (core + 1) * BPC)
    n1 = neigh1[rows].reshape(NCHUNK, CHUNK, NN1).transpose(1, 0, 2)
    n0 = neigh0[rows].reshape(NCHUNK, CHUNK, N0).transpose(1, 0, 2)
    iv = inputs[rows].reshape(NCHUNK, CHUNK, 1).transpose(1, 0, 2)
    return np.ascontiguousarray(
        np.concatenate([n1, n0, iv], axis=2).reshape(CHUNK, NCHUNK * IDXW)
    )


last_results = None  # test.py reads exec_time_ns off this


def kernel(inputs, neigh0, neigh1, embed_table, W1, W0, b0):
    global last_results
    inputs = np.asarray(inputs).astype(np.int32).reshape(B)
    neigh0 = np.asarray(neigh0).astype(np.int32).reshape(B, N0)
    neigh1 = np.asarray(neigh1).astype(np.int32).reshape(B, NN1)
    table = np.ascontiguousarray(
        np.asarray(embed_table, dtype=np.float32).astype(ml_dtypes.float8_e4m3)
    )
    W1 = np.ascontiguousarray(np.asarray(W1, dtype=np.float32))
    W0 = np.ascontiguousarray(np.asarray(W0, dtype=np.float32))
    b0 = np.ascontiguousarray(np.asarray(b0, dtype=np.float32).reshape(1, H0))

    if "nc" not in _prog_cache:
        _prog_cache["nc"] = _build_program()
    nc = _prog_cache["nc"]

    in_maps = [
        {
            "table": table,
            "idx": _make_idx(inputs, neigh0, neigh1, m),
            "w1": W1,
            "w0": W0,
            "b0": b0,
        }
        for m in range(N_CORES)
    ]
    trace = bool(os.environ.get("KERNEL_TRACE"))
    last_results = run_bass_kernel_spmd(
        nc, in_maps, list(range(N_CORES)), trace=trace
    )
    return np.concatenate(
        [last_results.results[m]["out"] for m in range(N_CORES)], axis=0
    )
